# revision 22
# baseline (speedup 1.0000x reference)
"""Trainium2 Bass kernel for nn_DeltaNet_22488448762128.

Full-input contract: kernel(**inputs) takes the unsharded numpy inputs and
returns the full [B, L, HID] output. Internally shards across 8 NeuronCores:
core = (b, hg) with b in {0,1} and hg in {0..3} head-groups of 4 heads.
Each core computes projections for its 4 heads, a chunked (C=128) linear
attention scan, and a partial output projection; the host sums the 4 partial
outputs per batch element.

Math (per head, chunk c of size C, state S aug with z column):
  a_t   = cumprod(beta) within chunk;  aC = a_{C-1}
  q~_t  = phi(rope(q))_t * a_t ;  k^_s = phi(rope(k))_s * aC / a_s
  A^T[s,t] = (phi_k_s . q~_t) * (1/a_s) * [s<=t]
  nu    = A^T.T @ [V|1] + q~ @ S_aug   ;  y_t = nu[:, :D] / (nu[:, D] + eps)
  S_aug = aC * S_aug + k^T @ [V|1]

Dtypes: projections and output projection run on the PE in float32r
(TF32-like fast path, 1 cyc/row at N=512); the intra-chunk scan matmuls run
in bf16 (inputs rounded once); the recurrent state S is carried in bf16 with
fp32 accumulation inside each op. Elementwise math is fp32.
"""

import math
import numpy as np

B, L, HID = 2, 2048, 2048
H, D = 16, 128
HG = 4              # heads per core
C = 128             # chunk size
NCHUNK = L // C     # 16
NK = HID // C       # 16 contraction tiles
EPS = 1e-6
BETA_MIN, BETA_MAX = 0.8, 0.9995
NCORES = 8
GW = HG * D         # 512, per-core projection width

_CACHE = {}


def _rope_tables():
    half = D // 2
    inv_freq = (1.0 / (10000.0 ** (np.arange(half, dtype=np.float32) /
                                   np.float32(half)))).astype(np.float32)
    t = np.arange(L, dtype=np.float32)
    freqs = t[:, None] * inv_freq[None, :]
    cos = np.cos(freqs).astype(np.float32)   # [L, 64]
    sin = np.sin(freqs).astype(np.float32)
    # chunk-major: [128, NCHUNK*64], block c = rows c*128..c*128+128
    def rearr(m):
        return np.ascontiguousarray(
            m.reshape(NCHUNK, C, half).transpose(1, 0, 2).reshape(C, NCHUNK * half))
    return rearr(cos), rearr(sin)


def _build(cfg):
    import concourse.bass as bass
    import concourse.bacc as bacc
    import concourse.tile as tile
    import concourse.mybir as mybir
    from contextlib import ExitStack

    dt = mybir.dt
    F32 = dt.float32
    BF16 = dt.bfloat16
    Alu = mybir.AluOpType
    Act = mybir.ActivationFunctionType
    half = D // 2

    use_f32r = cfg.get("f32r", True)
    use_bf16_scan = cfg.get("bf16_scan", False)
    nch = cfg.get("nchunk", NCHUNK)
    FR = dt.float32r if use_f32r else F32   # fast-matmul operand dtype
    SC = BF16 if use_bf16_scan else F32     # scan-matmul operand dtype

    nc = bacc.Bacc("TRN2", target_bir_lowering=False, debug=False,
                   enable_asserts=False, num_devices=NCORES)

    # ---- DRAM I/O (host passes PE-blocked layouts, see make_in_maps) ----
    xT_d = nc.dram_tensor("xTb", [NCHUNK, C, HID], FR, kind="ExternalInput").ap()
    wq_d = nc.dram_tensor("wq", [C, NK * GW], FR, kind="ExternalInput").ap()
    wk_d = nc.dram_tensor("wk", [C, NK * GW], FR, kind="ExternalInput").ap()
    wv_d = nc.dram_tensor("wv", [C, NK * GW], FR, kind="ExternalInput").ap()
    wg_d = nc.dram_tensor("wg", [C, NK * HG], FR, kind="ExternalInput").ap()
    wo_d = nc.dram_tensor("wo", [C, HG * HID], FR, kind="ExternalInput").ap()
    nbg_d = nc.dram_tensor("nbg4", [C, HG], F32, kind="ExternalInput").ap()
    cos_d = nc.dram_tensor("cosr", [C, NCHUNK * half], F32, kind="ExternalInput").ap()
    sin_d = nc.dram_tensor("sinr", [C, NCHUNK * half], F32, kind="ExternalInput").ap()
    mask_d = nc.dram_tensor("maskT", [C, C], F32, kind="ExternalInput").ap()
    id_d = nc.dram_tensor("ident", [C, C], F32, kind="ExternalInput").ap()
    out_d = nc.dram_tensor("out", [L, HID], F32, kind="ExternalOutput").ap()

    with ExitStack() as ctx:
        tc = ctx.enter_context(tile.TileContext(nc))

        cpool = ctx.enter_context(tc.tile_pool(name="consts", bufs=1))
        cos_t = cpool.tile([C, NCHUNK * half], F32, tag="cos")
        sin_t = cpool.tile([C, NCHUNK * half], F32, tag="sin")
        mask_t = cpool.tile([C, C], F32, tag="mask")
        id_t = cpool.tile([C, C], F32, tag="id")
        id_s = cpool.tile([C, C], SC, tag="id_s")
        ones_t = cpool.tile([C, C], F32, tag="ones")
        ones_s = cpool.tile([C, 1], SC, tag="ones_s")
        nbg_t = cpool.tile([C, HG], F32, tag="nbg")
        nc.vector.memset(ones_t[:], 1.0)
        nc.vector.memset(ones_s[:], 1.0)
        yt_all = cpool.tile([C, NCHUNK * HG * C], FR, tag="yta")  # [128, 8192]

        with ExitStack() as main:
            wpool = main.enter_context(tc.tile_pool(name="w", bufs=1))
            wq_t = wpool.tile([C, NK * GW], FR, tag="wq")
            wk_t = wpool.tile([C, NK * GW], FR, tag="wk")
            wv_t = wpool.tile([C, NK * GW], FR, tag="wv")
            wg_t = wpool.tile([C, NK * HG], FR, tag="wg")
            nc.sync.dma_start(wg_t[:], wg_d)
            for k in range(NK):
                nc.sync.dma_start(wq_t[:, bass.ts(k, GW)], wq_d[:, bass.ts(k, GW)])
                nc.sync.dma_start(wk_t[:, bass.ts(k, GW)], wk_d[:, bass.ts(k, GW)])
                nc.sync.dma_start(wv_t[:, bass.ts(k, GW)], wv_d[:, bass.ts(k, GW)])
            nc.sync.dma_start(cos_t[:], cos_d)
            nc.sync.dma_start(sin_t[:], sin_d)
            nc.sync.dma_start(mask_t[:], mask_d)
            nc.sync.dma_start(id_t[:], id_d)
            nc.sync.dma_start(nbg_t[:], nbg_d)
            nc.scalar.copy(id_s[:], id_t[:])

            # chunk-local SBUF pools
            xp = main.enter_context(tc.tile_pool(name="xp", bufs=cfg.get("xp", 2)))
            big2 = main.enter_context(tc.tile_pool(name="big2", bufs=2))
            big1 = main.enter_context(tc.tile_pool(name="big1", bufs=1))
            sml = main.enter_context(tc.tile_pool(name="sml", bufs=cfg.get("sml", 3)))
            spool = main.enter_context(tc.tile_pool(name="spool", bufs=2))

            # psum pools (8 banks total)
            pp = main.enter_context(tc.tile_pool(name="pp", bufs=1, space="PSUM"))
            psb = main.enter_context(tc.tile_pool(
                name="psb", bufs=cfg.get("psb", 1), space="PSUM"))
            pnu = main.enter_context(tc.tile_pool(
                name="pnu", bufs=cfg.get("pnu", 1), space="PSUM"))
            pyt = main.enter_context(tc.tile_pool(
                name="pyt", bufs=cfg.get("pyt", 1), space="PSUM"))

            S_cur = []
            for h in range(HG):
                s0 = spool.tile([C, D + 1], SC, tag=f"s{h}")
                nc.vector.memset(s0[:], 0.0)
                S_cur.append(s0)

            for c in range(nch):
                # -- projections --
                xtb = xp.tile([C, HID], FR, tag="xtb")
                nc.scalar.dma_start(xtb[:], xT_d[c])

                q_ps = pp.tile([C, GW], F32, tag="pq")
                k_ps = pp.tile([C, GW], F32, tag="pk")
                v_ps = pp.tile([C, GW], F32, tag="pv")
                g_ps = pp.tile([C, HG], F32, tag="psmall")
                for k in range(NK):
                    lhs = xtb[:, bass.ts(k, C)]
                    st, sp = (k == 0), (k == NK - 1)
                    nc.tensor.matmul(q_ps[:], lhs, wq_t[:, bass.ts(k, GW)],
                                     start=st, stop=sp)
                    nc.tensor.matmul(k_ps[:], lhs, wk_t[:, bass.ts(k, GW)],
                                     start=st, stop=sp)
                    nc.tensor.matmul(v_ps[:], lhs, wv_t[:, bass.ts(k, GW)],
                                     start=st, stop=sp)
                    nc.tensor.matmul(g_ps[:], lhs, wg_t[:, bass.ts(k, HG)],
                                     start=st, stop=sp)

                q_sb = big2.tile([C, GW], F32, tag="q")
                k_sb = big2.tile([C, GW], F32, tag="k")
                v_sb = big2.tile([C, GW], SC, tag="v")
                nc.scalar.copy(q_sb[:], q_ps[:])
                nc.scalar.copy(k_sb[:], k_ps[:])
                nc.scalar.copy(v_sb[:], v_ps[:])

                # -- beta pipeline: beta = clip(1/(1+exp(-(g+bg)))) --
                beta_sb = sml.tile([C, HG], F32, tag="beta")
                for h in range(HG):
                    # exp(-(g+bg)) = Exp(g * -1 + (-bg))
                    nc.scalar.activation(beta_sb[:, h:h + 1], g_ps[:, h:h + 1],
                                         Act.Exp, bias=nbg_t[:, h:h + 1],
                                         scale=-1.0)
                nc.vector.tensor_scalar_add(beta_sb[:], beta_sb[:], 1.0)
                nc.vector.reciprocal(beta_sb[:], beta_sb[:])
                nc.vector.tensor_scalar(out=beta_sb[:], in0=beta_sb[:],
                                        scalar1=BETA_MIN, scalar2=BETA_MAX,
                                        op0=Alu.max, op1=Alu.min)
                btp_ps = pp.tile([HG, C], F32, tag="psmall")
                nc.tensor.transpose(btp_ps[:], beta_sb[:], id_t[:])
                btp_sb = sml.tile([HG, C], F32, tag="btp")
                nc.scalar.copy(btp_sb[:], btp_ps[:])
                aT_sb = sml.tile([HG, C], F32, tag="aT")
                nc.vector.tensor_tensor_scan(
                    out=aT_sb[:], data0=btp_sb[:], data1=ones_t[0:HG, :],
                    initial=1.0, op0=Alu.mult, op1=Alu.mult)
                a_ps = pp.tile([C, HG], F32, tag="psmall")
                nc.tensor.transpose(a_ps[:], aT_sb[:], id_t[0:HG, 0:HG])
                a_sb = sml.tile([C, HG], F32, tag="a")
                nc.scalar.copy(a_sb[:], a_ps[:])
                ainv_sb = sml.tile([C, HG], F32, tag="ainv")
                nc.vector.reciprocal(ainv_sb[:], a_sb[:])
                diag4 = sml.tile([HG, HG], F32, tag="diag4")
                nc.vector.tensor_scalar(out=diag4[:], in0=id_t[0:HG, 0:HG],
                                        scalar1=aT_sb[:, C - 1:C], scalar2=None,
                                        op0=Alu.mult)
                acb_ps = pp.tile([C, HG], F32, tag="psmall")
                nc.tensor.matmul(acb_ps[:], ones_t[0:HG, :], diag4[:],
                                 start=True, stop=True)
                acb_sb = sml.tile([C, HG], F32, tag="acb")
                nc.scalar.copy(acb_sb[:], acb_ps[:])
                acdiv_sb = sml.tile([C, HG], F32, tag="acdiv")
                nc.vector.tensor_tensor(out=acdiv_sb[:], in0=ainv_sb[:],
                                        in1=acb_sb[:], op=Alu.mult)

                # -- rope (DVE; mins on GPSIMD) --
                def rope(eng, src, dst, nm):
                    se = src[:].rearrange("p (h d) -> p h d", h=HG)[:, :, 0:half]
                    so = src[:].rearrange("p (h d) -> p h d", h=HG)[:, :, half:D]
                    de = dst[:].rearrange("p (h d) -> p h d", h=HG)[:, :, 0:half]
                    do = dst[:].rearrange("p (h d) -> p h d", h=HG)[:, :, half:D]
                    cc = bass.AP(tensor=cos_t[:].tensor,
                                 offset=cos_t[:, bass.ts(c, half)].offset,
                                 ap=[cos_t[:].ap[0], [0, HG], [1, half]])
                    ss = bass.AP(tensor=sin_t[:].tensor,
                                 offset=sin_t[:, bass.ts(c, half)].offset,
                                 ap=[sin_t[:].ap[0], [0, HG], [1, half]])
                    tmp = big1.tile([C, GW], F32, tag="rtmp")
                    t1 = tmp[:].rearrange("p (h d) -> p h d", h=HG)[:, :, 0:half]
                    t2 = tmp[:].rearrange("p (h d) -> p h d", h=HG)[:, :, half:D]
                    eng.tensor_tensor(out=t1, in0=se, in1=cc, op=Alu.mult)
                    eng.tensor_tensor(out=t2, in0=so, in1=ss, op=Alu.mult)
                    eng.tensor_tensor(out=de, in0=t1, in1=t2, op=Alu.subtract)
                    eng.tensor_tensor(out=t1, in0=se, in1=ss, op=Alu.mult)
                    eng.tensor_tensor(out=t2, in0=so, in1=cc, op=Alu.mult)
                    eng.tensor_tensor(out=do, in0=t1, in1=t2, op=Alu.add)

                qr = big1.tile([C, GW], F32, tag="qr")
                kr = big1.tile([C, GW], F32, tag="kr")
                rope(nc.vector, q_sb, qr, "q")
                rope(nc.vector, k_sb, kr, "k")

                # -- phi = exp(min(x,0)) + relu(x) --
                def phi(src, nm, out_dt):
                    tm = big1.tile([C, GW], F32, tag="m")
                    nc.gpsimd.tensor_scalar_min(tm[:], src[:], 0.0)
                    te = big2.tile([C, GW], F32, tag="e")
                    nc.scalar.activation(te[:], tm[:], Act.Exp)
                    ph = big2.tile([C, GW], out_dt, tag=nm)
                    nc.vector.scalar_tensor_tensor(out=ph[:], in0=src[:],
                                                   scalar=0.0, in1=te[:],
                                                   op0=Alu.max, op1=Alu.add)
                    return ph

                phiq = phi(qr, "q", SC)
                phik = phi(kr, "k", SC)

                # -- scan per head --
                for h in range(HG):
                    hs = bass.ts(h, D)
                    khat = sml.tile([C, D], SC, tag="khat")
                    nc.vector.tensor_scalar_mul(khat[:], phik[:, hs],
                                                acdiv_sb[:, h:h + 1])

                    tp_ps = psb.tile([C, 2 * D], SC, tag="scanbT")
                    nc.tensor.transpose(tp_ps[:, 0:D], phiq[:, hs], id_s[:])
                    nc.tensor.transpose(tp_ps[:, D:2 * D], phik[:, hs], id_s[:])
                    qT_sb = sml.tile([C, D], SC, tag="qT")
                    kT_sb = sml.tile([C, D], SC, tag="kT")
                    nc.scalar.copy(qT_sb[:], tp_ps[:, 0:D])
                    nc.scalar.copy(kT_sb[:], tp_ps[:, D:2 * D])

                    a_ps2 = psb.tile([C, D], F32, tag="scanbA")
                    nc.tensor.matmul(a_ps2[:], kT_sb[:], qT_sb[:],
                                     start=True, stop=True)
                    A_sb = sml.tile([C, C], SC, tag="A")
                    nc.vector.scalar_tensor_tensor(
                        out=A_sb[:], in0=a_ps2[:],
                        scalar=ainv_sb[:, h:h + 1], in1=mask_t[:],
                        op0=Alu.mult, op1=Alu.mult)

                    nuU = pnu.tile([C, 2 * (D + 1)], F32, tag="nuU")
                    U = nuU[:, D + 1:2 * (D + 1)]
                    nu = nuU[:, 0:D + 1]
                    nc.tensor.matmul(U[:, 0:D], khat[:], v_sb[:, hs],
                                     start=True, stop=False)
                    nc.tensor.matmul(U[:, D:D + 1], khat[:], ones_s[:, 0:1],
                                     start=False, stop=False)
                    nc.tensor.matmul(nu[:, 0:D], A_sb[:], v_sb[:, hs],
                                     start=False, stop=False)
                    nc.tensor.matmul(nu[:, D:D + 1], A_sb[:], ones_s[:, 0:1],
                                     start=False, stop=False)
                    nc.tensor.matmul(nu[:], qT_sb[:], S_cur[h][:],
                                     start=False, stop=True)

                    S_new = spool.tile([C, D + 1], SC, tag=f"s{h}")
                    nc.vector.scalar_tensor_tensor(
                        out=S_new[:], in0=S_cur[h][:], scalar=acb_sb[:, h:h + 1],
                        in1=U, op0=Alu.mult, op1=Alu.add)
                    S_cur[h] = S_new

                    rd = sml.tile([C, 1], F32, tag="rd")
                    nc.vector.tensor_scalar_add(rd[:], nu[:, D:D + 1], EPS)
                    nc.vector.reciprocal(rd[:], rd[:])
                    y_sb = sml.tile([C, D], F32, tag="y")
                    nc.vector.tensor_scalar_mul(y_sb[:], nu[:, 0:D], rd[:])

                    ytp_ps = (psb.tile([C, 3 * D], F32, tag="scanb")
                              if cfg.get("pyt", 1) == 0 else
                              pyt.tile([C, D], F32, tag="ytp"))
                    nc.tensor.transpose(ytp_ps[:, 0:D], y_sb[:], id_t[:])
                    nc.scalar.copy(yt_all[:, bass.ts(c * HG + h, D)],
                                   ytp_ps[:, 0:D])

        # ---- output projection (final phase) ----
        with ExitStack() as fin:
            fpool = fin.enter_context(tc.tile_pool(name="fin", bufs=1))
            wo_t = fpool.tile([C, HG * HID], FR, tag="wo")
            for h in range(HG):
                nc.sync.dma_start(wo_t[:, bass.ts(h, HID)], wo_d[:, bass.ts(h, HID)])
            osb = fin.enter_context(tc.tile_pool(name="osb", bufs=4))
            pout = fin.enter_context(tc.tile_pool(name="pout", bufs=3, space="PSUM"))

            NO = HID // GW  # 4 output col tiles
            for c in range(nch):
                for o in range(NO):
                    out_ps = pout.tile([C, GW], F32, tag="po")
                    for h in range(HG):
                        nc.tensor.matmul(
                            out_ps[:],
                            yt_all[:, bass.ts(c * HG + h, D)],
                            wo_t[:, h * HID + o * GW:h * HID + (o + 1) * GW],
                            start=(h == 0), stop=(h == HG - 1))
                    out_sb = osb.tile([C, GW], F32, tag="osb")
                    if o % 2 == 0:
                        nc.scalar.copy(out_sb[:], out_ps[:])
                    else:
                        nc.vector.tensor_copy(out_sb[:], out_ps[:])
                    nc.sync.dma_start(out_d[bass.ts(c, C), bass.ts(o, GW)],
                                       out_sb[:])

    nc.compile()
    return nc


def _get_nc(cfg_key="default", **cfg):
    if cfg_key not in _CACHE:
        _CACHE[cfg_key] = _build(cfg)
    return _CACHE[cfg_key]


def _block_w(W, fw):
    # [HID, fw] -> [C, NK*fw] with block k = W[k*128:(k+1)*128, :]
    return np.ascontiguousarray(
        W.reshape(NK, C, fw).transpose(1, 0, 2).reshape(C, NK * fw)).astype(np.float32)


def make_in_maps(x, Wq, Wk, Wv, Wg, bg, Wo, bo):
    cosr, sinr = _rope_tables()
    maskT = np.triu(np.ones((C, C), np.float32))
    ident = np.eye(C, dtype=np.float32)
    x = np.asarray(x, np.float32)
    Wq, Wk, Wv = np.asarray(Wq), np.asarray(Wk), np.asarray(Wv)
    Wg, bg, Wo, bo = np.asarray(Wg), np.asarray(bg), np.asarray(Wo), np.asarray(bo)
    in_maps = []
    for core in range(NCORES):
        b, hg = divmod(core, 4)
        cs = slice(hg * GW, (hg + 1) * GW)
        hsl = slice(hg * HG, (hg + 1) * HG)
        # xTb[c, p, k*128+f] = x[b][c*128+f, k*128+p]
        xTb = np.ascontiguousarray(
            x[b].reshape(NCHUNK, C, NK, C).transpose(0, 3, 2, 1)
            .reshape(NCHUNK, C, HID)).astype(np.float32)
        in_maps.append({
            "xTb": xTb,
            "wq": _block_w(Wq[:, cs], GW),
            "wk": _block_w(Wk[:, cs], GW),
            "wv": _block_w(Wv[:, cs], GW),
            "wg": _block_w(Wg[:, hsl], HG),
            "wo": np.ascontiguousarray(
                Wo[cs, :].reshape(HG, C, HID).transpose(1, 0, 2)
                .reshape(C, HG * HID)).astype(np.float32),
            "nbg4": np.tile(-bg[None, hsl], (C, 1)).astype(np.float32),
            "cosr": cosr, "sinr": sinr,
            "maskT": maskT, "ident": ident,
        })
    return in_maps


def kernel(x, Wq, Wk, Wv, Wg, bg, Wo, bo, _trace=False, **cfg):
    from concourse.bass_utils import run_bass_kernel_spmd
    nc = _get_nc(**cfg)
    in_maps = make_in_maps(x, Wq, Wk, Wv, Wg, bg, Wo, bo)
    res = run_bass_kernel_spmd(nc, in_maps, core_ids=list(range(NCORES)),
                               trace=_trace)
    out = np.zeros((B, L, HID), np.float32)
    for core in range(NCORES):
        b = core // 4
        out[b] += res.results[core]["out"]
    out += np.asarray(bo, np.float32)[None, None, :]
    kernel._last_results = res
    return out


# revision 24
# speedup vs baseline: 1.0183x; 1.0183x over previous
"""Trainium2 Bass kernel for nn_DeltaNet_22488448762128.

Full-input contract: kernel(**inputs) takes the unsharded numpy inputs and
returns the full [B, L, HID] output. Internally shards across 8 NeuronCores:
core = (b, hg) with b in {0,1} and hg in {0..3} head-groups of 4 heads.
Each core computes projections for its 4 heads, a chunked (C=128) linear
attention scan, and a partial output projection; the host sums the 4 partial
outputs per batch element.

Math (per head, chunk c of size C, state S aug with z column):
  a_t   = cumprod(beta) within chunk;  aC = a_{C-1}
  q~_t  = phi(rope(q))_t * a_t ;  k^_s = phi(rope(k))_s * aC / a_s
  A^T[s,t] = (phi_k_s . q~_t) * (1/a_s) * [s<=t]
  nu    = A^T.T @ [V|1] + q~ @ S_aug   ;  y_t = nu[:, :D] / (nu[:, D] + eps)
  S_aug = aC * S_aug + k^T @ [V|1]

Dtypes: projections and output projection run on the PE in float32r
(TF32-like fast path, 1 cyc/row at N=512); the intra-chunk scan matmuls run
in bf16 (inputs rounded once); the recurrent state S is carried in bf16 with
fp32 accumulation inside each op. Elementwise math is fp32.
"""

import math
import numpy as np

B, L, HID = 2, 2048, 2048
H, D = 16, 128
HG = 4              # heads per core
C = 128             # chunk size
NCHUNK = L // C     # 16
NK = HID // C       # 16 contraction tiles
EPS = 1e-6
BETA_MIN, BETA_MAX = 0.8, 0.9995
NCORES = 8
GW = HG * D         # 512, per-core projection width

_CACHE = {}


def _rope_tables():
    half = D // 2
    inv_freq = (1.0 / (10000.0 ** (np.arange(half, dtype=np.float32) /
                                   np.float32(half)))).astype(np.float32)
    t = np.arange(L, dtype=np.float32)
    freqs = t[:, None] * inv_freq[None, :]
    cos = np.cos(freqs).astype(np.float32)   # [L, 64]
    sin = np.sin(freqs).astype(np.float32)
    # chunk-major: [128, NCHUNK*64], block c = rows c*128..c*128+128
    def rearr(m):
        return np.ascontiguousarray(
            m.reshape(NCHUNK, C, half).transpose(1, 0, 2).reshape(C, NCHUNK * half))
    return rearr(cos), rearr(sin)


def _build(cfg):
    import concourse.bass as bass
    import concourse.bacc as bacc
    import concourse.tile as tile
    import concourse.mybir as mybir
    from contextlib import ExitStack

    dt = mybir.dt
    F32 = dt.float32
    BF16 = dt.bfloat16
    Alu = mybir.AluOpType
    Act = mybir.ActivationFunctionType
    half = D // 2

    use_f32r = cfg.get("f32r", True)
    use_bf16_scan = cfg.get("bf16_scan", False)
    use_bf16_A = cfg.get("bf16A", True)
    nch = cfg.get("nchunk", NCHUNK)
    FR = dt.float32r if use_f32r else F32   # fast-matmul operand dtype
    SC = BF16 if use_bf16_scan else F32     # scan-matmul operand dtype

    nc = bacc.Bacc("TRN2", target_bir_lowering=False, debug=False,
                   enable_asserts=False, num_devices=NCORES)

    # ---- DRAM I/O (host passes PE-blocked layouts, see make_in_maps) ----
    xT_d = nc.dram_tensor("xTb", [NCHUNK, C, HID], FR, kind="ExternalInput").ap()
    wq_d = nc.dram_tensor("wq", [C, NK * GW], FR, kind="ExternalInput").ap()
    wk_d = nc.dram_tensor("wk", [C, NK * GW], FR, kind="ExternalInput").ap()
    wv_d = nc.dram_tensor("wv", [C, NK * GW], FR, kind="ExternalInput").ap()
    wg_d = nc.dram_tensor("wg", [C, NK * HG], FR, kind="ExternalInput").ap()
    wo_d = nc.dram_tensor("wo", [C, HG * HID], FR, kind="ExternalInput").ap()
    nbg_d = nc.dram_tensor("nbg4", [C, HG], F32, kind="ExternalInput").ap()
    cos_d = nc.dram_tensor("cosr", [C, NCHUNK * half], F32, kind="ExternalInput").ap()
    sin_d = nc.dram_tensor("sinr", [C, NCHUNK * half], F32, kind="ExternalInput").ap()
    mask_d = nc.dram_tensor("maskT", [C, C], F32, kind="ExternalInput").ap()
    id_d = nc.dram_tensor("ident", [C, C], F32, kind="ExternalInput").ap()
    out_d = nc.dram_tensor("out", [L, HID], F32, kind="ExternalOutput").ap()

    with ExitStack() as ctx:
        tc = ctx.enter_context(tile.TileContext(nc))

        cpool = ctx.enter_context(tc.tile_pool(name="consts", bufs=1))
        cos_t = cpool.tile([C, NCHUNK * half], F32, tag="cos")
        sin_t = cpool.tile([C, NCHUNK * half], F32, tag="sin")
        mask_t = cpool.tile([C, C], F32, tag="mask")
        id_t = cpool.tile([C, C], F32, tag="id")
        id_s = cpool.tile([C, C], SC, tag="id_s")
        ones_t = cpool.tile([C, C], F32, tag="ones")
        ones_s = cpool.tile([C, 1], SC, tag="ones_s")
        nbg_t = cpool.tile([C, HG], F32, tag="nbg")
        nc.vector.memset(ones_t[:], 1.0)
        nc.vector.memset(ones_s[:], 1.0)
        yt_all = cpool.tile([C, NCHUNK * HG * C], FR, tag="yta")  # [128, 8192]

        with ExitStack() as main:
            wpool = main.enter_context(tc.tile_pool(name="w", bufs=1))
            wq_t = wpool.tile([C, NK * GW], FR, tag="wq")
            wk_t = wpool.tile([C, NK * GW], FR, tag="wk")
            wv_t = wpool.tile([C, NK * GW], FR, tag="wv")
            wg_t = wpool.tile([C, NK * HG], FR, tag="wg")
            nc.sync.dma_start(wg_t[:], wg_d)
            for k in range(NK):
                nc.sync.dma_start(wq_t[:, bass.ts(k, GW)], wq_d[:, bass.ts(k, GW)])
                nc.sync.dma_start(wk_t[:, bass.ts(k, GW)], wk_d[:, bass.ts(k, GW)])
                nc.sync.dma_start(wv_t[:, bass.ts(k, GW)], wv_d[:, bass.ts(k, GW)])
            nc.sync.dma_start(cos_t[:], cos_d)
            nc.sync.dma_start(sin_t[:], sin_d)
            nc.sync.dma_start(mask_t[:], mask_d)
            nc.sync.dma_start(id_t[:], id_d)
            nc.sync.dma_start(nbg_t[:], nbg_d)
            nc.scalar.copy(id_s[:], id_t[:])

            # chunk-local SBUF pools
            xp = main.enter_context(tc.tile_pool(name="xp", bufs=cfg.get("xp", 2)))
            big2 = main.enter_context(tc.tile_pool(name="big2", bufs=2))
            big1 = main.enter_context(tc.tile_pool(name="big1", bufs=1))
            sml = main.enter_context(tc.tile_pool(name="sml", bufs=cfg.get("sml", 3)))
            spool = main.enter_context(tc.tile_pool(name="spool", bufs=2))

            # psum pools (8 banks total)
            pp = main.enter_context(tc.tile_pool(name="pp", bufs=1, space="PSUM"))
            psb = main.enter_context(tc.tile_pool(
                name="psb", bufs=cfg.get("psb", 1), space="PSUM"))
            pnu = main.enter_context(tc.tile_pool(
                name="pnu", bufs=cfg.get("pnu", 1), space="PSUM"))
            pyt = main.enter_context(tc.tile_pool(
                name="pyt", bufs=cfg.get("pyt", 1), space="PSUM"))

            S_cur = []
            for h in range(HG):
                s0 = spool.tile([C, D + 1], SC, tag=f"s{h}")
                nc.vector.memset(s0[:], 0.0)
                S_cur.append(s0)

            for c in range(nch):
                # -- projections --
                xtb = xp.tile([C, HID], FR, tag="xtb")
                nc.scalar.dma_start(xtb[:], xT_d[c])

                q_ps = pp.tile([C, GW], F32, tag="pq")
                k_ps = pp.tile([C, GW], F32, tag="pk")
                v_ps = pp.tile([C, GW], F32, tag="pv")
                g_ps = pp.tile([C, HG], F32, tag="psmall")
                for k in range(NK):
                    lhs = xtb[:, bass.ts(k, C)]
                    st, sp = (k == 0), (k == NK - 1)
                    nc.tensor.matmul(q_ps[:], lhs, wq_t[:, bass.ts(k, GW)],
                                     start=st, stop=sp)
                    nc.tensor.matmul(k_ps[:], lhs, wk_t[:, bass.ts(k, GW)],
                                     start=st, stop=sp)
                    nc.tensor.matmul(v_ps[:], lhs, wv_t[:, bass.ts(k, GW)],
                                     start=st, stop=sp)
                    nc.tensor.matmul(g_ps[:], lhs, wg_t[:, bass.ts(k, HG)],
                                     start=st, stop=sp)

                q_sb = big2.tile([C, GW], F32, tag="q")
                k_sb = big2.tile([C, GW], F32, tag="k")
                v_sb = big2.tile([C, HG * (D + 1)], SC, tag="v")
                nc.scalar.copy(q_sb[:], q_ps[:])
                nc.scalar.copy(k_sb[:], k_ps[:])
                v_aug = v_sb[:].rearrange("p (h e) -> p h e", e=D + 1)
                nc.scalar.copy(v_aug[:, :, 0:D],
                               v_ps[:].rearrange("p (h e) -> p h e", e=D))
                nc.vector.memset(v_aug[:, :, D:D + 1], 1.0)

                # -- beta pipeline: beta = clip(1/(1+exp(-(g+bg)))) --
                beta_sb = sml.tile([C, HG], F32, tag="beta")
                for h in range(HG):
                    # exp(-(g+bg)) = Exp(g * -1 + (-bg))
                    nc.scalar.activation(beta_sb[:, h:h + 1], g_ps[:, h:h + 1],
                                         Act.Exp, bias=nbg_t[:, h:h + 1],
                                         scale=-1.0)
                nc.vector.tensor_scalar_add(beta_sb[:], beta_sb[:], 1.0)
                nc.vector.reciprocal(beta_sb[:], beta_sb[:])
                nc.vector.tensor_scalar(out=beta_sb[:], in0=beta_sb[:],
                                        scalar1=BETA_MIN, scalar2=BETA_MAX,
                                        op0=Alu.max, op1=Alu.min)
                btp_ps = pp.tile([HG, C], F32, tag="psmall")
                nc.tensor.transpose(btp_ps[:], beta_sb[:], id_t[:])
                btp_sb = sml.tile([HG, C], F32, tag="btp")
                nc.scalar.copy(btp_sb[:], btp_ps[:])
                aT_sb = sml.tile([HG, C], F32, tag="aT")
                nc.vector.tensor_tensor_scan(
                    out=aT_sb[:], data0=btp_sb[:], data1=ones_t[0:HG, :],
                    initial=1.0, op0=Alu.mult, op1=Alu.mult)
                a_ps = pp.tile([C, HG], F32, tag="psmall")
                nc.tensor.transpose(a_ps[:], aT_sb[:], id_t[0:HG, 0:HG])
                a_sb = sml.tile([C, HG], F32, tag="a")
                nc.scalar.copy(a_sb[:], a_ps[:])
                ainv_sb = sml.tile([C, HG], F32, tag="ainv")
                nc.vector.reciprocal(ainv_sb[:], a_sb[:])
                diag4 = sml.tile([HG, HG], F32, tag="diag4")
                nc.vector.tensor_scalar(out=diag4[:], in0=id_t[0:HG, 0:HG],
                                        scalar1=aT_sb[:, C - 1:C], scalar2=None,
                                        op0=Alu.mult)
                acb_ps = pp.tile([C, HG], F32, tag="psmall")
                nc.tensor.matmul(acb_ps[:], ones_t[0:HG, :], diag4[:],
                                 start=True, stop=True)
                acb_sb = sml.tile([C, HG], F32, tag="acb")
                nc.scalar.copy(acb_sb[:], acb_ps[:])
                acdiv_sb = sml.tile([C, HG], F32, tag="acdiv")
                nc.vector.tensor_tensor(out=acdiv_sb[:], in0=ainv_sb[:],
                                        in1=acb_sb[:], op=Alu.mult)

                # -- rope (DVE; mins on GPSIMD) --
                def rope(eng, src, dst, nm):
                    se = src[:].rearrange("p (h d) -> p h d", h=HG)[:, :, 0:half]
                    so = src[:].rearrange("p (h d) -> p h d", h=HG)[:, :, half:D]
                    de = dst[:].rearrange("p (h d) -> p h d", h=HG)[:, :, 0:half]
                    do = dst[:].rearrange("p (h d) -> p h d", h=HG)[:, :, half:D]
                    cc = bass.AP(tensor=cos_t[:].tensor,
                                 offset=cos_t[:, bass.ts(c, half)].offset,
                                 ap=[cos_t[:].ap[0], [0, HG], [1, half]])
                    ss = bass.AP(tensor=sin_t[:].tensor,
                                 offset=sin_t[:, bass.ts(c, half)].offset,
                                 ap=[sin_t[:].ap[0], [0, HG], [1, half]])
                    tmp = big1.tile([C, GW], F32, tag="rtmp")
                    t1 = tmp[:].rearrange("p (h d) -> p h d", h=HG)[:, :, 0:half]
                    t2 = tmp[:].rearrange("p (h d) -> p h d", h=HG)[:, :, half:D]
                    eng.tensor_tensor(out=t1, in0=se, in1=cc, op=Alu.mult)
                    eng.tensor_tensor(out=t2, in0=so, in1=ss, op=Alu.mult)
                    eng.tensor_tensor(out=de, in0=t1, in1=t2, op=Alu.subtract)
                    eng.tensor_tensor(out=t1, in0=se, in1=ss, op=Alu.mult)
                    eng.tensor_tensor(out=t2, in0=so, in1=cc, op=Alu.mult)
                    eng.tensor_tensor(out=do, in0=t1, in1=t2, op=Alu.add)

                qr = big1.tile([C, GW], F32, tag="qr")
                kr = big1.tile([C, GW], F32, tag="kr")
                rope(nc.vector, q_sb, qr, "q")
                rope(nc.vector, k_sb, kr, "k")

                # -- phi = exp(min(x,0)) + relu(x) --
                def phi(src, nm, out_dt):
                    tm = big1.tile([C, GW], F32, tag="m")
                    nc.gpsimd.tensor_scalar_min(tm[:], src[:], 0.0)
                    te = big2.tile([C, GW], F32, tag="e")
                    nc.scalar.activation(te[:], tm[:], Act.Exp)
                    ph = big2.tile([C, GW], out_dt, tag=nm)
                    nc.vector.scalar_tensor_tensor(out=ph[:], in0=src[:],
                                                   scalar=0.0, in1=te[:],
                                                   op0=Alu.max, op1=Alu.add)
                    return ph

                phiq = phi(qr, "q", SC)
                phik = phi(kr, "k", SC)

                # -- scan per head --
                for h in range(HG):
                    hs = bass.ts(h, D)
                    khat = sml.tile([C, D], SC, tag="khat")
                    nc.vector.tensor_scalar_mul(khat[:], phik[:, hs],
                                                acdiv_sb[:, h:h + 1])

                    AD = BF16 if use_bf16_A else SC
                    tp_ps = psb.tile([C, 2 * D], SC, tag="scanbT")
                    nc.tensor.transpose(tp_ps[:, 0:D], phiq[:, hs], id_s[:])
                    nc.tensor.transpose(tp_ps[:, D:2 * D], phik[:, hs], id_s[:])
                    qT_sb = sml.tile([C, D], SC, tag="qT")
                    nc.scalar.copy(qT_sb[:], tp_ps[:, 0:D])
                    qT_a = qT_sb
                    if use_bf16_A:
                        qT_a = sml.tile([C, D], AD, tag="qTa")
                        nc.scalar.copy(qT_a[:], tp_ps[:, 0:D])
                    kT_sb = sml.tile([C, D], AD, tag="kT")
                    nc.scalar.copy(kT_sb[:], tp_ps[:, D:2 * D])

                    a_ps2 = psb.tile([C, D], F32, tag="scanbA")
                    nc.tensor.matmul(a_ps2[:], kT_sb[:], qT_a[:],
                                     start=True, stop=True)
                    A_sb = sml.tile([C, C], SC, tag="A")
                    nc.vector.scalar_tensor_tensor(
                        out=A_sb[:], in0=a_ps2[:],
                        scalar=ainv_sb[:, h:h + 1], in1=mask_t[:],
                        op0=Alu.mult, op1=Alu.mult)

                    nuU = pnu.tile([C, 2 * (D + 1)], F32, tag="nuU")
                    U = nuU[:, D + 1:2 * (D + 1)]
                    nu = nuU[:, 0:D + 1]
                    vh = v_sb[:, h * (D + 1):(h + 1) * (D + 1)]
                    nc.tensor.matmul(U[:], khat[:], vh, start=True, stop=False)
                    nc.tensor.matmul(nu[:], A_sb[:], vh, start=False, stop=False)
                    nc.tensor.matmul(nu[:], qT_sb[:], S_cur[h][:],
                                     start=False, stop=True)

                    S_new = spool.tile([C, D + 1], SC, tag=f"s{h}")
                    nc.vector.scalar_tensor_tensor(
                        out=S_new[:], in0=S_cur[h][:], scalar=acb_sb[:, h:h + 1],
                        in1=U, op0=Alu.mult, op1=Alu.add)
                    S_cur[h] = S_new

                    rd = sml.tile([C, 1], F32, tag="rd")
                    nc.vector.tensor_scalar_add(rd[:], nu[:, D:D + 1], EPS)
                    nc.vector.reciprocal(rd[:], rd[:])
                    y_sb = sml.tile([C, D], F32, tag="y")
                    nc.vector.tensor_scalar_mul(y_sb[:], nu[:, 0:D], rd[:])

                    ytp_ps = (psb.tile([C, 3 * D], F32, tag="scanb")
                              if cfg.get("pyt", 1) == 0 else
                              pyt.tile([C, D], F32, tag="ytp"))
                    nc.tensor.transpose(ytp_ps[:, 0:D], y_sb[:], id_t[:])
                    nc.scalar.copy(yt_all[:, bass.ts(c * HG + h, D)],
                                   ytp_ps[:, 0:D])

        # ---- output projection (final phase) ----
        with ExitStack() as fin:
            fpool = fin.enter_context(tc.tile_pool(name="fin", bufs=1))
            wo_t = fpool.tile([C, HG * HID], FR, tag="wo")
            for h in range(HG):
                nc.sync.dma_start(wo_t[:, bass.ts(h, HID)], wo_d[:, bass.ts(h, HID)])
            osb = fin.enter_context(tc.tile_pool(name="osb", bufs=4))
            pout = fin.enter_context(tc.tile_pool(name="pout", bufs=3, space="PSUM"))

            NO = HID // GW  # 4 output col tiles
            for c in range(nch):
                for o in range(NO):
                    out_ps = pout.tile([C, GW], F32, tag="po")
                    for h in range(HG):
                        nc.tensor.matmul(
                            out_ps[:],
                            yt_all[:, bass.ts(c * HG + h, D)],
                            wo_t[:, h * HID + o * GW:h * HID + (o + 1) * GW],
                            start=(h == 0), stop=(h == HG - 1))
                    out_sb = osb.tile([C, GW], F32, tag="osb")
                    if o % 2 == 0:
                        nc.scalar.copy(out_sb[:], out_ps[:])
                    else:
                        nc.vector.tensor_copy(out_sb[:], out_ps[:])
                    nc.sync.dma_start(out_d[bass.ts(c, C), bass.ts(o, GW)],
                                       out_sb[:])

    nc.compile()
    return nc


def _get_nc(cfg_key="default", **cfg):
    if cfg_key not in _CACHE:
        _CACHE[cfg_key] = _build(cfg)
    return _CACHE[cfg_key]


def _block_w(W, fw):
    # [HID, fw] -> [C, NK*fw] with block k = W[k*128:(k+1)*128, :]
    return np.ascontiguousarray(
        W.reshape(NK, C, fw).transpose(1, 0, 2).reshape(C, NK * fw)).astype(np.float32)


def make_in_maps(x, Wq, Wk, Wv, Wg, bg, Wo, bo):
    cosr, sinr = _rope_tables()
    maskT = np.triu(np.ones((C, C), np.float32))
    ident = np.eye(C, dtype=np.float32)
    x = np.asarray(x, np.float32)
    Wq, Wk, Wv = np.asarray(Wq), np.asarray(Wk), np.asarray(Wv)
    Wg, bg, Wo, bo = np.asarray(Wg), np.asarray(bg), np.asarray(Wo), np.asarray(bo)
    in_maps = []
    for core in range(NCORES):
        b, hg = divmod(core, 4)
        cs = slice(hg * GW, (hg + 1) * GW)
        hsl = slice(hg * HG, (hg + 1) * HG)
        # xTb[c, p, k*128+f] = x[b][c*128+f, k*128+p]
        xTb = np.ascontiguousarray(
            x[b].reshape(NCHUNK, C, NK, C).transpose(0, 3, 2, 1)
            .reshape(NCHUNK, C, HID)).astype(np.float32)
        in_maps.append({
            "xTb": xTb,
            "wq": _block_w(Wq[:, cs], GW),
            "wk": _block_w(Wk[:, cs], GW),
            "wv": _block_w(Wv[:, cs], GW),
            "wg": _block_w(Wg[:, hsl], HG),
            "wo": np.ascontiguousarray(
                Wo[cs, :].reshape(HG, C, HID).transpose(1, 0, 2)
                .reshape(C, HG * HID)).astype(np.float32),
            "nbg4": np.tile(-bg[None, hsl], (C, 1)).astype(np.float32),
            "cosr": cosr, "sinr": sinr,
            "maskT": maskT, "ident": ident,
        })
    return in_maps


def kernel(x, Wq, Wk, Wv, Wg, bg, Wo, bo, _trace=False, **cfg):
    from concourse.bass_utils import run_bass_kernel_spmd
    nc = _get_nc(**cfg)
    in_maps = make_in_maps(x, Wq, Wk, Wv, Wg, bg, Wo, bo)
    res = run_bass_kernel_spmd(nc, in_maps, core_ids=list(range(NCORES)),
                               trace=_trace)
    out = np.zeros((B, L, HID), np.float32)
    for core in range(NCORES):
        b = core // 4
        out[b] += res.results[core]["out"]
    out += np.asarray(bo, np.float32)[None, None, :]
    kernel._last_results = res
    return out


# revision 26
# speedup vs baseline: 1.0326x; 1.0141x over previous
"""Trainium2 Bass kernel for nn_DeltaNet_22488448762128.

Full-input contract: kernel(**inputs) takes the unsharded numpy inputs and
returns the full [B, L, HID] output. Internally shards across 8 NeuronCores:
core = (b, hg) with b in {0,1} and hg in {0..3} head-groups of 4 heads.
Each core computes projections for its 4 heads, a chunked (C=128) linear
attention scan, and a partial output projection; the host sums the 4 partial
outputs per batch element.

Math (per head, chunk c of size C, state S aug with z column):
  a_t   = cumprod(beta) within chunk;  aC = a_{C-1}
  q~_t  = phi(rope(q))_t * a_t ;  k^_s = phi(rope(k))_s * aC / a_s
  A^T[s,t] = (phi_k_s . q~_t) * (1/a_s) * [s<=t]
  nu    = A^T.T @ [V|1] + q~ @ S_aug   ;  y_t = nu[:, :D] / (nu[:, D] + eps)
  S_aug = aC * S_aug + k^T @ [V|1]

Dtypes: projections and output projection run on the PE in float32r
(TF32-like fast path, 1 cyc/row at N=512). In the scan, only the A-matmul
(intra-chunk attention weights) runs in bf16 — its rounding largely cancels
between numerator and denominator — while V, S and all accumulation paths
stay fp32. Measured end-to-end vs the fp32 reference: ~2.2e-4 relative.
"""

import math
import numpy as np

B, L, HID = 2, 2048, 2048
H, D = 16, 128
HG = 4              # heads per core
C = 128             # chunk size
NCHUNK = L // C     # 16
NK = HID // C       # 16 contraction tiles
EPS = 1e-6
BETA_MIN, BETA_MAX = 0.8, 0.9995
NCORES = 8
GW = HG * D         # 512, per-core projection width

_CACHE = {}


def _rope_tables():
    half = D // 2
    inv_freq = (1.0 / (10000.0 ** (np.arange(half, dtype=np.float32) /
                                   np.float32(half)))).astype(np.float32)
    t = np.arange(L, dtype=np.float32)
    freqs = t[:, None] * inv_freq[None, :]
    cos = np.cos(freqs).astype(np.float32)   # [L, 64]
    sin = np.sin(freqs).astype(np.float32)
    # chunk-major: [128, NCHUNK*64], block c = rows c*128..c*128+128
    def rearr(m):
        return np.ascontiguousarray(
            m.reshape(NCHUNK, C, half).transpose(1, 0, 2).reshape(C, NCHUNK * half))
    return rearr(cos), rearr(sin)


def _build(cfg):
    import concourse.bass as bass
    import concourse.bacc as bacc
    import concourse.tile as tile
    import concourse.mybir as mybir
    from contextlib import ExitStack

    dt = mybir.dt
    F32 = dt.float32
    BF16 = dt.bfloat16
    Alu = mybir.AluOpType
    Act = mybir.ActivationFunctionType
    half = D // 2

    use_f32r = cfg.get("f32r", True)
    use_bf16_scan = cfg.get("bf16_scan", False)
    use_bf16_A = cfg.get("bf16A", True)
    nch = cfg.get("nchunk", NCHUNK)
    FR = dt.float32r if use_f32r else F32   # fast-matmul operand dtype
    SC = BF16 if use_bf16_scan else F32     # scan-matmul operand dtype

    nc = bacc.Bacc("TRN2", target_bir_lowering=False, debug=False,
                   enable_asserts=False, num_devices=NCORES)

    # ---- DRAM I/O (host passes PE-blocked layouts, see make_in_maps) ----
    xT_d = nc.dram_tensor("xTb", [NCHUNK, C, HID], FR, kind="ExternalInput").ap()
    wq_d = nc.dram_tensor("wq", [C, NK * GW], FR, kind="ExternalInput").ap()
    wk_d = nc.dram_tensor("wk", [C, NK * GW], FR, kind="ExternalInput").ap()
    wv_d = nc.dram_tensor("wv", [C, NK * GW], FR, kind="ExternalInput").ap()
    wg_d = nc.dram_tensor("wg", [C, NK * HG], FR, kind="ExternalInput").ap()
    wo_d = nc.dram_tensor("wo", [C, HG * HID], FR, kind="ExternalInput").ap()
    nbg_d = nc.dram_tensor("nbg4", [C, HG], F32, kind="ExternalInput").ap()
    cos_d = nc.dram_tensor("cosr", [C, NCHUNK * half], F32, kind="ExternalInput").ap()
    sin_d = nc.dram_tensor("sinr", [C, NCHUNK * half], F32, kind="ExternalInput").ap()
    mask_d = nc.dram_tensor("maskT", [C, C], F32, kind="ExternalInput").ap()
    id_d = nc.dram_tensor("ident", [C, C], F32, kind="ExternalInput").ap()
    out_d = nc.dram_tensor("out", [L, HID], F32, kind="ExternalOutput").ap()

    with ExitStack() as ctx:
        tc = ctx.enter_context(tile.TileContext(nc))

        cpool = ctx.enter_context(tc.tile_pool(name="consts", bufs=1))
        cos_t = cpool.tile([C, NCHUNK * half], F32, tag="cos")
        sin_t = cpool.tile([C, NCHUNK * half], F32, tag="sin")
        mask_t = cpool.tile([C, C], F32, tag="mask")
        id_t = cpool.tile([C, C], F32, tag="id")
        id_s = cpool.tile([C, C], SC, tag="id_s")
        id_fr = cpool.tile([C, C], FR, tag="id_fr")
        ones_t = cpool.tile([C, C], F32, tag="ones")
        ones_s = cpool.tile([C, 1], SC, tag="ones_s")
        nbg_t = cpool.tile([C, HG], F32, tag="nbg")
        nc.vector.memset(ones_t[:], 1.0)
        nc.vector.memset(ones_s[:], 1.0)
        yt_all = cpool.tile([C, NCHUNK * HG * C], FR, tag="yta")  # [128, 8192]

        with ExitStack() as main:
            wpool = main.enter_context(tc.tile_pool(name="w", bufs=1))
            wq_t = wpool.tile([C, NK * GW], FR, tag="wq")
            wk_t = wpool.tile([C, NK * GW], FR, tag="wk")
            wv_t = wpool.tile([C, NK * GW], FR, tag="wv")
            wg_t = wpool.tile([C, NK * HG], FR, tag="wg")
            nc.sync.dma_start(wg_t[:], wg_d)
            for k in range(NK):
                nc.sync.dma_start(wq_t[:, bass.ts(k, GW)], wq_d[:, bass.ts(k, GW)])
                nc.sync.dma_start(wk_t[:, bass.ts(k, GW)], wk_d[:, bass.ts(k, GW)])
                nc.sync.dma_start(wv_t[:, bass.ts(k, GW)], wv_d[:, bass.ts(k, GW)])
            nc.sync.dma_start(cos_t[:], cos_d)
            nc.sync.dma_start(sin_t[:], sin_d)
            nc.sync.dma_start(mask_t[:], mask_d)
            nc.sync.dma_start(id_t[:], id_d)
            nc.sync.dma_start(nbg_t[:], nbg_d)
            nc.scalar.copy(id_s[:], id_t[:])
            nc.scalar.copy(id_fr[:], id_t[:])

            # chunk-local SBUF pools
            xp = main.enter_context(tc.tile_pool(name="xp", bufs=cfg.get("xp", 2)))
            big2 = main.enter_context(tc.tile_pool(name="big2", bufs=2))
            big1 = main.enter_context(tc.tile_pool(name="big1", bufs=1))
            sml = main.enter_context(tc.tile_pool(name="sml", bufs=cfg.get("sml", 3)))
            spool = main.enter_context(tc.tile_pool(name="spool", bufs=2))

            # psum pools (8 banks total)
            pp = main.enter_context(tc.tile_pool(name="pp", bufs=1, space="PSUM"))
            psb = main.enter_context(tc.tile_pool(
                name="psb", bufs=cfg.get("psb", 1), space="PSUM"))
            pnu = main.enter_context(tc.tile_pool(
                name="pnu", bufs=cfg.get("pnu", 1), space="PSUM"))
            pyt = main.enter_context(tc.tile_pool(
                name="pyt", bufs=cfg.get("pyt", 1), space="PSUM"))

            S_cur = []
            for h in range(HG):
                s0 = spool.tile([C, D + 1], SC, tag=f"s{h}")
                nc.vector.memset(s0[:], 0.0)
                S_cur.append(s0)

            for c in range(nch):
                # -- projections --
                xtb = xp.tile([C, HID], FR, tag="xtb")
                nc.scalar.dma_start(xtb[:], xT_d[c])

                q_ps = pp.tile([C, GW], F32, tag="pq")
                k_ps = pp.tile([C, GW], F32, tag="pk")
                v_ps = pp.tile([C, GW], F32, tag="pv")
                g_ps = pp.tile([C, HG], F32, tag="psmall")
                for k in range(NK):
                    lhs = xtb[:, bass.ts(k, C)]
                    st, sp = (k == 0), (k == NK - 1)
                    nc.tensor.matmul(q_ps[:], lhs, wq_t[:, bass.ts(k, GW)],
                                     start=st, stop=sp)
                    nc.tensor.matmul(k_ps[:], lhs, wk_t[:, bass.ts(k, GW)],
                                     start=st, stop=sp)
                    nc.tensor.matmul(v_ps[:], lhs, wv_t[:, bass.ts(k, GW)],
                                     start=st, stop=sp)
                    nc.tensor.matmul(g_ps[:], lhs, wg_t[:, bass.ts(k, HG)],
                                     start=st, stop=sp)

                q_sb = big2.tile([C, GW], F32, tag="q")
                k_sb = big2.tile([C, GW], F32, tag="k")
                v_sb = big2.tile([C, HG * (D + 1)], SC, tag="v")
                nc.scalar.copy(q_sb[:], q_ps[:])
                nc.scalar.copy(k_sb[:], k_ps[:])
                v_aug = v_sb[:].rearrange("p (h e) -> p h e", e=D + 1)
                nc.scalar.copy(v_aug[:, :, 0:D],
                               v_ps[:].rearrange("p (h e) -> p h e", e=D))
                nc.vector.memset(v_aug[:, :, D:D + 1], 1.0)

                # -- beta pipeline: beta = clip(1/(1+exp(-(g+bg)))) --
                beta_sb = sml.tile([C, HG], F32, tag="beta")
                for h in range(HG):
                    # exp(-(g+bg)) = Exp(g * -1 + (-bg))
                    nc.scalar.activation(beta_sb[:, h:h + 1], g_ps[:, h:h + 1],
                                         Act.Exp, bias=nbg_t[:, h:h + 1],
                                         scale=-1.0)
                nc.vector.tensor_scalar_add(beta_sb[:], beta_sb[:], 1.0)
                nc.vector.reciprocal(beta_sb[:], beta_sb[:])
                nc.vector.tensor_scalar(out=beta_sb[:], in0=beta_sb[:],
                                        scalar1=BETA_MIN, scalar2=BETA_MAX,
                                        op0=Alu.max, op1=Alu.min)
                btp_ps = pp.tile([HG, C], F32, tag="psmall")
                nc.tensor.transpose(btp_ps[:], beta_sb[:], id_t[:])
                btp_sb = sml.tile([HG, C], F32, tag="btp")
                nc.scalar.copy(btp_sb[:], btp_ps[:])
                aT_sb = sml.tile([HG, C], F32, tag="aT")
                nc.vector.tensor_tensor_scan(
                    out=aT_sb[:], data0=btp_sb[:], data1=ones_t[0:HG, :],
                    initial=1.0, op0=Alu.mult, op1=Alu.mult)
                a_ps = pp.tile([C, HG], F32, tag="psmall")
                nc.tensor.transpose(a_ps[:], aT_sb[:], id_t[0:HG, 0:HG])
                a_sb = sml.tile([C, HG], F32, tag="a")
                nc.scalar.copy(a_sb[:], a_ps[:])
                ainv_sb = sml.tile([C, HG], F32, tag="ainv")
                nc.vector.reciprocal(ainv_sb[:], a_sb[:])
                diag4 = sml.tile([HG, HG], F32, tag="diag4")
                nc.vector.tensor_scalar(out=diag4[:], in0=id_t[0:HG, 0:HG],
                                        scalar1=aT_sb[:, C - 1:C], scalar2=None,
                                        op0=Alu.mult)
                acb_ps = pp.tile([C, HG], F32, tag="psmall")
                nc.tensor.matmul(acb_ps[:], ones_t[0:HG, :], diag4[:],
                                 start=True, stop=True)
                acb_sb = sml.tile([C, HG], F32, tag="acb")
                nc.scalar.copy(acb_sb[:], acb_ps[:])
                acdiv_sb = sml.tile([C, HG], F32, tag="acdiv")
                nc.vector.tensor_tensor(out=acdiv_sb[:], in0=ainv_sb[:],
                                        in1=acb_sb[:], op=Alu.mult)

                # -- rope (DVE; mins on GPSIMD) --
                def rope(eng, src, dst, nm):
                    se = src[:].rearrange("p (h d) -> p h d", h=HG)[:, :, 0:half]
                    so = src[:].rearrange("p (h d) -> p h d", h=HG)[:, :, half:D]
                    de = dst[:].rearrange("p (h d) -> p h d", h=HG)[:, :, 0:half]
                    do = dst[:].rearrange("p (h d) -> p h d", h=HG)[:, :, half:D]
                    cc = bass.AP(tensor=cos_t[:].tensor,
                                 offset=cos_t[:, bass.ts(c, half)].offset,
                                 ap=[cos_t[:].ap[0], [0, HG], [1, half]])
                    ss = bass.AP(tensor=sin_t[:].tensor,
                                 offset=sin_t[:, bass.ts(c, half)].offset,
                                 ap=[sin_t[:].ap[0], [0, HG], [1, half]])
                    tmp = big1.tile([C, GW], F32, tag="rtmp")
                    t1 = tmp[:].rearrange("p (h d) -> p h d", h=HG)[:, :, 0:half]
                    t2 = tmp[:].rearrange("p (h d) -> p h d", h=HG)[:, :, half:D]
                    eng.tensor_tensor(out=t1, in0=se, in1=cc, op=Alu.mult)
                    eng.tensor_tensor(out=t2, in0=so, in1=ss, op=Alu.mult)
                    eng.tensor_tensor(out=de, in0=t1, in1=t2, op=Alu.subtract)
                    eng.tensor_tensor(out=t1, in0=se, in1=ss, op=Alu.mult)
                    eng.tensor_tensor(out=t2, in0=so, in1=cc, op=Alu.mult)
                    eng.tensor_tensor(out=do, in0=t1, in1=t2, op=Alu.add)

                qr = big1.tile([C, GW], F32, tag="qr")
                kr = big1.tile([C, GW], F32, tag="kr")
                rope(nc.vector, q_sb, qr, "q")
                rope(nc.vector, k_sb, kr, "k")

                # -- phi = exp(min(x,0)) + relu(x) --
                def phi(src, nm, out_dt):
                    tm = big1.tile([C, GW], F32, tag="m")
                    nc.gpsimd.tensor_scalar_min(tm[:], src[:], 0.0)
                    te = big2.tile([C, GW], F32, tag="e")
                    nc.scalar.activation(te[:], tm[:], Act.Exp)
                    ph = big2.tile([C, GW], out_dt, tag=nm)
                    nc.vector.scalar_tensor_tensor(out=ph[:], in0=src[:],
                                                   scalar=0.0, in1=te[:],
                                                   op0=Alu.max, op1=Alu.add)
                    return ph

                phiq = phi(qr, "q", FR)
                phik = phi(kr, "k", FR)

                # -- scan per head --
                for h in range(HG):
                    hs = bass.ts(h, D)
                    khat = sml.tile([C, D], SC, tag="khat")
                    nc.vector.tensor_scalar_mul(khat[:], phik[:, hs],
                                                acdiv_sb[:, h:h + 1])

                    AD = BF16 if use_bf16_A else SC
                    tp_ps = psb.tile([C, 2 * D], FR, tag="scanbT")
                    nc.tensor.transpose(tp_ps[:, 0:D], phiq[:, hs], id_fr[:])
                    nc.tensor.transpose(tp_ps[:, D:2 * D], phik[:, hs], id_fr[:])
                    qT_sb = sml.tile([C, D], SC, tag="qT")
                    nc.scalar.copy(qT_sb[:], tp_ps[:, 0:D])
                    qT_a = qT_sb
                    if use_bf16_A:
                        qT_a = sml.tile([C, D], AD, tag="qTa")
                        nc.scalar.copy(qT_a[:], tp_ps[:, 0:D])
                    kT_sb = sml.tile([C, D], AD, tag="kT")
                    nc.scalar.copy(kT_sb[:], tp_ps[:, D:2 * D])

                    a_ps2 = psb.tile([C, D], F32, tag="scanbA")
                    nc.tensor.matmul(a_ps2[:], kT_sb[:], qT_a[:],
                                     start=True, stop=True)
                    A_sb = sml.tile([C, C], SC, tag="A")
                    nc.vector.scalar_tensor_tensor(
                        out=A_sb[:], in0=a_ps2[:],
                        scalar=ainv_sb[:, h:h + 1], in1=mask_t[:],
                        op0=Alu.mult, op1=Alu.mult)

                    nuU = pnu.tile([C, 2 * (D + 1)], F32, tag="nuU")
                    U = nuU[:, D + 1:2 * (D + 1)]
                    nu = nuU[:, 0:D + 1]
                    vh = v_sb[:, h * (D + 1):(h + 1) * (D + 1)]
                    nc.tensor.matmul(U[:], khat[:], vh, start=True, stop=False)
                    nc.tensor.matmul(nu[:], A_sb[:], vh, start=False, stop=False)
                    nc.tensor.matmul(nu[:], qT_sb[:], S_cur[h][:],
                                     start=False, stop=True)

                    S_new = spool.tile([C, D + 1], SC, tag=f"s{h}")
                    nc.vector.scalar_tensor_tensor(
                        out=S_new[:], in0=S_cur[h][:], scalar=acb_sb[:, h:h + 1],
                        in1=U, op0=Alu.mult, op1=Alu.add)
                    S_cur[h] = S_new

                    rd = sml.tile([C, 1], F32, tag="rd")
                    nc.vector.tensor_scalar_add(rd[:], nu[:, D:D + 1], EPS)
                    nc.vector.reciprocal(rd[:], rd[:])
                    y_sb = sml.tile([C, D], FR, tag="y")
                    nc.vector.tensor_scalar_mul(y_sb[:], nu[:, 0:D], rd[:])

                    ytp_ps = pyt.tile([C, D], FR, tag="ytp")
                    nc.tensor.transpose(ytp_ps[:, 0:D], y_sb[:], id_fr[:])
                    nc.scalar.copy(yt_all[:, bass.ts(c * HG + h, D)],
                                   ytp_ps[:, 0:D])

        # ---- output projection (final phase) ----
        with ExitStack() as fin:
            fpool = fin.enter_context(tc.tile_pool(name="fin", bufs=1))
            wo_t = fpool.tile([C, HG * HID], FR, tag="wo")
            for h in range(HG):
                nc.sync.dma_start(wo_t[:, bass.ts(h, HID)], wo_d[:, bass.ts(h, HID)])
            osb = fin.enter_context(tc.tile_pool(name="osb", bufs=4))
            pout = fin.enter_context(tc.tile_pool(name="pout", bufs=3, space="PSUM"))

            NO = HID // GW  # 4 output col tiles
            for c in range(nch):
                for o in range(NO):
                    out_ps = pout.tile([C, GW], F32, tag="po")
                    for h in range(HG):
                        nc.tensor.matmul(
                            out_ps[:],
                            yt_all[:, bass.ts(c * HG + h, D)],
                            wo_t[:, h * HID + o * GW:h * HID + (o + 1) * GW],
                            start=(h == 0), stop=(h == HG - 1))
                    out_sb = osb.tile([C, GW], F32, tag="osb")
                    if o % 2 == 0:
                        nc.scalar.copy(out_sb[:], out_ps[:])
                    else:
                        nc.vector.tensor_copy(out_sb[:], out_ps[:])
                    nc.sync.dma_start(out_d[bass.ts(c, C), bass.ts(o, GW)],
                                       out_sb[:])

    nc.compile()
    return nc


def _get_nc(cfg_key="default", **cfg):
    if cfg_key not in _CACHE:
        _CACHE[cfg_key] = _build(cfg)
    return _CACHE[cfg_key]


def _block_w(W, fw):
    # [HID, fw] -> [C, NK*fw] with block k = W[k*128:(k+1)*128, :]
    return np.ascontiguousarray(
        W.reshape(NK, C, fw).transpose(1, 0, 2).reshape(C, NK * fw)).astype(np.float32)


def make_in_maps(x, Wq, Wk, Wv, Wg, bg, Wo, bo):
    cosr, sinr = _rope_tables()
    maskT = np.triu(np.ones((C, C), np.float32))
    ident = np.eye(C, dtype=np.float32)
    x = np.asarray(x, np.float32)
    Wq, Wk, Wv = np.asarray(Wq), np.asarray(Wk), np.asarray(Wv)
    Wg, bg, Wo, bo = np.asarray(Wg), np.asarray(bg), np.asarray(Wo), np.asarray(bo)
    in_maps = []
    for core in range(NCORES):
        b, hg = divmod(core, 4)
        cs = slice(hg * GW, (hg + 1) * GW)
        hsl = slice(hg * HG, (hg + 1) * HG)
        # xTb[c, p, k*128+f] = x[b][c*128+f, k*128+p]
        xTb = np.ascontiguousarray(
            x[b].reshape(NCHUNK, C, NK, C).transpose(0, 3, 2, 1)
            .reshape(NCHUNK, C, HID)).astype(np.float32)
        in_maps.append({
            "xTb": xTb,
            "wq": _block_w(Wq[:, cs], GW),
            "wk": _block_w(Wk[:, cs], GW),
            "wv": _block_w(Wv[:, cs], GW),
            "wg": _block_w(Wg[:, hsl], HG),
            "wo": np.ascontiguousarray(
                Wo[cs, :].reshape(HG, C, HID).transpose(1, 0, 2)
                .reshape(C, HG * HID)).astype(np.float32),
            "nbg4": np.tile(-bg[None, hsl], (C, 1)).astype(np.float32),
            "cosr": cosr, "sinr": sinr,
            "maskT": maskT, "ident": ident,
        })
    return in_maps


def kernel(x, Wq, Wk, Wv, Wg, bg, Wo, bo, _trace=False, **cfg):
    from concourse.bass_utils import run_bass_kernel_spmd
    nc = _get_nc(**cfg)
    in_maps = make_in_maps(x, Wq, Wk, Wv, Wg, bg, Wo, bo)
    res = run_bass_kernel_spmd(nc, in_maps, core_ids=list(range(NCORES)),
                               trace=_trace)
    out = np.zeros((B, L, HID), np.float32)
    for core in range(NCORES):
        b = core // 4
        out[b] += res.results[core]["out"]
    out += np.asarray(bo, np.float32)[None, None, :]
    kernel._last_results = res
    return out
